# revision 26
# baseline (speedup 1.0000x reference)
"""Trainium2 Bass kernel for nn_BeliefDecoder (LSTM decoder with categorical
sampling), data-parallel over 8 NeuronCores.

Contract: kernel(**inputs) takes FULL unsharded inputs (as produced by
setup_inputs()) and returns the FULL output tuple
(logits (B, 15, 26) f32, samples (B, 15) int32).

Strategy
--------
- Pure data parallel: batch 65536 -> 8 cores x 8192 rows; weights replicated.
- Sampling must be bit-identical to jax.random.categorical: Gumbel noise is
  precomputed on host (CPU jax, threefry -> bit-exact) and shipped to the
  device. jax.random.categorical(k, logits) == argmax(gumbel(k, shape) +
  logits).
- On device, everything runs in a transposed layout [feature/vocab partitions,
  batch free]:
    h0 = tanh(ctx @ Wp + bp)                      (PE + ACT)
    per step: gates = Wh.h + EW'[tok] + xc + b    (PE accumulate + DVE add)
              i,f,g,o nonlinearities              (ACT)
              c,h update                          (DVE)
              logits = Wo.h                       (PE)
              z = logits + G[t]                   (DVE)
              m = max over vocab partitions       (GPSIMD partition_all_reduce)
              onehot = (z == m)                   (DVE is_equal)
  where xc = ctx @ Wi[:256] is precomputed once per batch-tile (context is
  time-invariant) and EW' = embed_table @ Wi[256:] + bh folds the embedding
  lookup into a tiny 26-row matmul against the onehot (the sampled token feeds
  back without ever materialising embeddings).
- The samples themselves are recovered on the host from the logits the kernel
  already outputs: argmax(logits + G) in f32 is bit-identical to the device's
  (z == m) selection.
- Matmuls run in true fp32 (4-pass H/L) so the sampled trajectory tracks the
  f32 reference closely enough that argmax flips are rare.
"""

import os
import numpy as np

H = 256          # hidden
T = 15           # decode steps (num_components)
V = 26           # vocab
E = 64           # embed dim
B = 65536        # batch
NCORES = 8
BT = 256         # batch tile (moving free dim per matmul)
BLOC = B // NCORES
NTILES = BLOC // BT  # 32

_CACHE = {}
LAST_RESULTS = None


def build_nc(n_tiles, use_f32r=False):
    """Build the Bass/Tile program for one core handling n_tiles*BT rows."""
    import concourse.bass as bass
    import concourse.tile as tile
    import concourse.mybir as mybir
    from concourse import bacc

    f32 = mybir.dt.float32
    mmdt = mybir.dt.float32r if use_f32r else mybir.dt.float32
    Sig = mybir.ActivationFunctionType.Sigmoid
    Tanh = mybir.ActivationFunctionType.Tanh
    ADD = mybir.AluOpType.add
    ISEQ = mybir.AluOpType.is_equal
    bloc = n_tiles * BT

    nc = bacc.Bacc("TRN2", target_bir_lowering=False, debug=False)

    ctxT = nc.dram_tensor("ctxT", (H, bloc), f32, kind="ExternalInput")
    gumb = nc.dram_tensor("gumb", (n_tiles, 128, 2, T, V), f32, kind="ExternalInput")
    wp_d = nc.dram_tensor("wp", (H, H), f32, kind="ExternalInput")
    wi_d = nc.dram_tensor("wi1", (H, 4 * H), f32, kind="ExternalInput")
    wh_d = nc.dram_tensor("wh", (H, 4 * H), f32, kind="ExternalInput")
    wo_d = nc.dram_tensor("wo", (H, V), f32, kind="ExternalInput")
    ew_d = nc.dram_tensor("ew", (V, 4 * H), f32, kind="ExternalInput")
    cst_d = nc.dram_tensor("consts", (128, 10), f32, kind="ExternalInput")
    id_d = nc.dram_tensor("ident", (128, 128), f32, kind="ExternalInput")
    louts = nc.dram_tensor(
        "louts", (n_tiles, 128, 2, T, V), f32, kind="ExternalOutput"
    )

    with tile.TileContext(nc) as tc:
        with (
            tc.tile_pool(name="weights", bufs=1) as wpool,
            tc.tile_pool(name="state", bufs=3) as spool,
            tc.tile_pool(name="work", bufs=2) as kpool,
            tc.tile_pool(name="oh", bufs=4) as ohpool,
            tc.tile_pool(name="stage", bufs=2) as stpool,
            tc.tile_pool(name="pgates", bufs=1, space="PSUM") as pg_pool,
            tc.tile_pool(name="plog", bufs=1, space="PSUM") as pl_pool,
            tc.tile_pool(name="pbc", bufs=1, space="PSUM") as pbc_pool,
            tc.tile_pool(name="psetup", bufs=1, space="PSUM") as ps_pool,
        ):
            # ---- load weights (once) ----
            wp_sb = wpool.tile([128, 2, 2, 128], mmdt, tag="wp")
            nc.sync.dma_start(
                wp_sb[:], wp_d.rearrange("(k p) (m c) -> p k m c", p=128, c=128)
            )
            wi_sb = wpool.tile([128, 2, 8, 128], mmdt, tag="wi")
            nc.sync.dma_start(
                wi_sb[:], wi_d.rearrange("(k p) (m c) -> p k m c", p=128, c=128)
            )
            wh_sb = wpool.tile([128, 2, 8, 128], mmdt, tag="wh")
            nc.sync.dma_start(
                wh_sb[:], wh_d.rearrange("(k p) (m c) -> p k m c", p=128, c=128)
            )
            wo_sb = wpool.tile([128, 2, V], mmdt, tag="wo")
            nc.sync.dma_start(wo_sb[:], wo_d.rearrange("(k p) v -> p k v", p=128))
            ew_sb = wpool.tile([V, 8, 128], mmdt, tag="ew")
            nc.sync.dma_start(ew_sb[:], ew_d.rearrange("v (m c) -> v m c", c=128))
            cst = wpool.tile([128, 10], f32, tag="cst")
            nc.sync.dma_start(cst[:], cst_d[:, :])
            id_sb = wpool.tile([128, 128], f32, tag="ident")
            nc.sync.dma_start(id_sb[:], id_d[:, :])

            state = {}  # per-tile persistent tiles
            prev_oh = {}

            def setup(j):
                ctx_sb = kpool.tile([128, 2, BT], mmdt, tag="ctx")
                nc.sync.dma_start(
                    ctx_sb[:],
                    ctxT.rearrange("(k p) b -> p k b", p=128)[
                        :, :, j * BT : (j + 1) * BT
                    ],
                )
                g_sb = stpool.tile([128, 2, T, V], f32, tag="gum")
                nc.sync.dma_start(g_sb[:], gumb[j])

                # h0 = tanh(ctx @ Wp + bp)
                hT = spool.tile([128, 2, BT], mmdt, tag="h")
                cT = spool.tile([128, 2, BT], f32, tag="c")
                nc.vector.memset(cT[:], 0.0)
                p_wp = ps_pool.tile([128, 2, BT], f32, tag="psetup")
                for m in range(2):
                    for k in range(2):
                        nc.tensor.matmul(
                            p_wp[:, m, :], wp_sb[:, k, m, :], ctx_sb[:, k, :],
                            start=(k == 0), stop=(k == 1),
                        )
                for m in range(2):
                    nc.scalar.activation(
                        hT[:, m, :], p_wp[:, m, :], Tanh, bias=cst[:, 8 + m : 9 + m]
                    )

                # xc = ctx @ Wi[:256]  (time-invariant gate contribution)
                xc = spool.tile([128, 8, BT], f32, tag="xc")
                for half in range(2):
                    p_xc = ps_pool.tile([128, 4, BT], f32, tag="psetup")
                    for mm in range(4):
                        m = half * 4 + mm
                        for k in range(2):
                            nc.tensor.matmul(
                                p_xc[:, mm, :], wi_sb[:, k, m, :], ctx_sb[:, k, :],
                                start=(k == 0), stop=(k == 1),
                            )
                    nc.scalar.copy(xc[:, half * 4 : half * 4 + 4, :], p_xc[:])

                lstage = stpool.tile([128, 2, T, V], f32, tag="lst")
                state[j] = (hT, cT, xc, g_sb, lstage)

            def step(j, t):
                hT, cT, xc, g_sb, lstage = state[j]
                # gates = Wh.h (+ EW'[tok_prev]) ; psum bank b holds gate b
                p_g = pg_pool.tile([128, 8, BT], f32, tag="pg")
                for gate in range(4):
                    for c2 in range(2):
                        m = gate * 2 + c2
                        for k in range(2):
                            nc.tensor.matmul(
                                p_g[:, m, :], wh_sb[:, k, m, :], hT[:, k, :],
                                start=(c2 == 0 and k == 0),
                                stop=(t == 0 and c2 == 1 and k == 1),
                            )
                if t > 0:
                    oh = prev_oh[j]
                    for gate in range(4):
                        for c2 in range(2):
                            m = gate * 2 + c2
                            nc.tensor.matmul(
                                p_g[:, m, :], ew_sb[:, m, :], oh[:],
                                start=False, stop=(c2 == 1),
                            )

                # gates += xc (+ bias); then nonlinearities
                gsb = kpool.tile([128, 8, BT], f32, tag="gsb")
                if t == 0:
                    # bias0 = bh + start_embed @ Wi[256:]
                    for ch in range(8):
                        nc.vector.scalar_tensor_tensor(
                            gsb[:, ch, :], p_g[:, ch, :], cst[:, ch : ch + 1],
                            xc[:, ch, :], op0=ADD, op1=ADD,
                        )
                else:
                    nc.vector.tensor_add(gsb[:], p_g[:], xc[:])

                acts = kpool.tile([128, 8, BT], f32, tag="acts")
                for gate in range(4):
                    nc.scalar.activation(
                        acts[:, 2 * gate : 2 * gate + 2, :],
                        gsb[:, 2 * gate : 2 * gate + 2, :],
                        Tanh if gate == 2 else Sig,
                    )

                # c = sig(f)*c + sig(i)*tanh(g);  h = sig(o)*tanh(c)
                t1 = kpool.tile([128, 2, BT], f32, tag="t1")
                t2 = kpool.tile([128, 2, BT], f32, tag="t2")
                nc.vector.tensor_mul(t1[:], acts[:, 2:4, :], cT[:])
                nc.vector.tensor_mul(t2[:], acts[:, 0:2, :], acts[:, 4:6, :])
                nc.vector.tensor_add(cT[:], t1[:], t2[:])
                tct = kpool.tile([128, 2, BT], f32, tag="tct")
                nc.scalar.activation(tct[:], cT[:], Tanh)
                nc.vector.tensor_mul(hT[:], acts[:, 6:8, :], tct[:])

                # logits in BATCH-major: out[batch 128, 26] via hT-as-stationary
                p_lb = pl_pool.tile([128, 2, V], f32, tag="plb")
                for c in range(2):
                    for k in range(2):
                        nc.tensor.matmul(
                            p_lb[:, c, :],
                            hT[:, k, c * 128 : (c + 1) * 128],
                            wo_sb[:, k, :],
                            start=(k == 0), stop=(k == 1),
                        )
                nc.scalar.copy(lstage[:, :, t, :], p_lb[:])

                if t < T - 1:
                    # z = logits + gumbel'; argmax along the free (vocab) dim;
                    # onehot back to vocab-major via PE transpose for feedback.
                    zt = kpool.tile([128, 2, V], f32, tag="zt")
                    nc.vector.tensor_add(zt[:], p_lb[:], g_sb[:, :, t, :])
                    mt = kpool.tile([128, 2], f32, tag="mt")
                    nc.vector.tensor_reduce(
                        mt[:], zt[:], axis=mybir.AxisListType.X,
                        op=mybir.AluOpType.max,
                    )
                    eqt = kpool.tile([128, 2, V], f32, tag="eqt")
                    for c in range(2):
                        nc.vector.tensor_scalar(
                            eqt[:, c, :], zt[:, c, :], mt[:, c : c + 1], None,
                            op0=ISEQ,
                        )
                    p_oh = pbc_pool.tile([V, 2, 128], f32, tag="poh")
                    for c in range(2):
                        nc.tensor.transpose(p_oh[:, c, :], eqt[:, c, :], id_sb[:])
                    oh = ohpool.tile([V, BT], mmdt, tag="oh")
                    nc.scalar.copy(oh[:], p_oh[:])
                    prev_oh[j] = oh

            def finish(j):
                lstage = state[j][4]
                nc.sync.dma_start(louts[j], lstage[:])
                del state[j]
                prev_oh.pop(j, None)

            for jj in range(0, n_tiles, 2):
                pair = [jj] if jj + 1 >= n_tiles else [jj, jj + 1]
                for j in pair:
                    setup(j)
                for t in range(T):
                    for j in pair:
                        step(j, t)
                for j in pair:
                    finish(j)

    nc.compile()
    return nc


def _prep_host(context, embed_table, start_embed, Wp, bp, Wi, Wh, bh, Wo, bo, seed):
    """Host-side preprocessing: gumbel noise, weight repacks, per-core shards."""
    import jax
    import jax.numpy as jnp

    cpu = jax.devices("cpu")[0]
    with jax.default_device(cpu):
        keys = jax.random.split(jax.random.key(int(seed)), T)
        # per-key gumbel calls: bit-exact with jax.random.categorical's
        # internal noise (NOTE: vmap over keys yields different bits!)
        gfn = jax.jit(lambda k: jax.random.gumbel(k, (B, V), jnp.float32))
        G = np.stack([np.asarray(gfn(keys[t])) for t in range(T)])  # (T, B, V)
    bo = np.asarray(bo, np.float32)
    Gp = (G + bo[None, None, :]).astype(np.float32)  # z = logits + (gumbel + bo)

    Wi = np.asarray(Wi)
    bh = np.asarray(bh)
    ew = (
        np.asarray(embed_table).astype(np.float64) @ Wi[H:].astype(np.float64)
        + bh.astype(np.float64)
    ).astype(np.float32)  # (V, 4H)
    bias0 = (
        np.asarray(start_embed).astype(np.float64) @ Wi[H:].astype(np.float64)
        + bh.astype(np.float64)
    ).astype(np.float32)  # (4H,)
    consts = np.zeros((128, 10), np.float32)
    for ch in range(8):
        consts[:, ch] = bias0[ch * 128 : (ch + 1) * 128]
    bp = np.asarray(bp, np.float32)
    consts[:, 8] = bp[:128]
    consts[:, 9] = bp[128:]

    context = np.asarray(context, np.float32)
    in_maps = []
    for c in range(NCORES):
        sl = slice(c * BLOC, (c + 1) * BLOC)
        ctxT = np.ascontiguousarray(context[sl].T)  # (H, BLOC)
        # (T, BLOC, V) -> (NTILES, 128, 2, T, V); b = j*256 + c*128 + p
        gc = np.ascontiguousarray(
            Gp[:, sl, :].reshape(T, NTILES, 2, 128, V).transpose(1, 3, 2, 0, 4)
        )
        in_maps.append(
            dict(
                ctxT=ctxT,
                gumb=gc,
                wp=np.asarray(Wp, np.float32),
                wi1=Wi[:H].astype(np.float32),
                wh=np.asarray(Wh, np.float32),
                wo=np.asarray(Wo, np.float32),
                ew=ew,
                consts=consts,
                ident=np.eye(128, dtype=np.float32),
            )
        )
    return in_maps, Gp, bo


def _get_runner():
    """Build (once) a cached sharded-PJRT executable for the kernel, modeled on
    concourse.bass2jax.run_bass_via_pjrt's multi-core path, plus a bench()
    that times steady-state execution with device-resident inputs."""
    if "runner" in _CACHE:
        return _CACHE["runner"]
    import jax
    import jax.numpy as jnp
    from jax.sharding import Mesh, PartitionSpec, NamedSharding
    from jax.experimental.shard_map import shard_map
    import concourse.mybir as mybir
    from concourse import bass2jax

    nc = _CACHE.get(("nc", NTILES))
    if nc is None:
        nc = _CACHE[("nc", NTILES)] = build_nc(NTILES)
    bass2jax.install_neuronx_cc_hook()

    partition_name = (
        nc.partition_id_tensor.name if nc.partition_id_tensor else None
    )
    in_names, out_names, out_avals, zero_shapes = [], [], [], []
    for alloc in nc.m.functions[0].allocations:
        if not isinstance(alloc, mybir.MemoryLocationSet):
            continue
        name = alloc.memorylocations[0].name
        if alloc.kind == "ExternalInput":
            if name != partition_name:
                in_names.append(name)
        elif alloc.kind == "ExternalOutput":
            shape = tuple(alloc.tensor_shape)
            dtype = mybir.dt.np(alloc.dtype)
            out_names.append(name)
            out_avals.append(jax.core.ShapedArray(shape, dtype))
            zero_shapes.append((shape, dtype))
    n_params = len(in_names)
    n_outs = len(out_names)
    all_names = tuple(
        in_names + out_names + ([partition_name] if partition_name else [])
    )

    def _body(*args):
        operands = list(args)
        if partition_name is not None:
            operands.append(bass2jax.partition_id_tensor())
        outs = bass2jax._bass_exec_p.bind(
            *operands,
            out_avals=tuple(out_avals),
            in_names=all_names,
            out_names=tuple(out_names),
            lowering_input_output_aliases=(),
            sim_require_finite=True,
            sim_require_nnan=True,
            nc=nc,
        )
        return tuple(outs)

    devices = jax.devices()[:NCORES]
    mesh = Mesh(np.asarray(devices), ("core",))
    pspec = PartitionSpec("core")
    sharded = jax.jit(
        shard_map(
            _body,
            mesh=mesh,
            in_specs=(pspec,) * (n_params + n_outs),
            out_specs=(pspec,) * n_outs,
            check_rep=False,
        ),
        donate_argnums=tuple(range(n_params, n_params + n_outs)),
        keep_unused=True,
    )
    zeros_fn = jax.jit(
        lambda: tuple(
            jnp.zeros((NCORES * s[0], *s[1:]), d) for s, d in zero_shapes
        ),
        out_shardings=tuple(NamedSharding(mesh, pspec) for _ in zero_shapes),
    )

    def run(in_maps):
        concat_in = [
            np.concatenate([m[name] for m in in_maps], axis=0) for name in in_names
        ]
        dev_in = [jax.device_put(a, NamedSharding(mesh, pspec)) for a in concat_in]
        out = sharded(*dev_in, *zeros_fn())
        results = []
        for c in range(NCORES):
            results.append(
                {
                    name: np.asarray(out[i]).reshape(NCORES, *out_avals[i].shape)[c]
                    for i, name in enumerate(out_names)
                }
            )
        return results, dev_in

    def bench(dev_in, iters=3):
        import time

        times = []
        for _ in range(iters):
            zs = jax.block_until_ready(zeros_fn())
            t0 = time.perf_counter()
            out = sharded(*dev_in, *zs)
            jax.block_until_ready(out)
            times.append(time.perf_counter() - t0)
        return times

    _CACHE["runner"] = (run, bench)
    return _CACHE["runner"]


def bench_exec(iters=3):
    """Steady-state wall time per execution (device-resident inputs)."""
    run, bench = _get_runner()
    dev_in = _CACHE.get("last_dev_in")
    if dev_in is None:
        raise RuntimeError("call kernel() first")
    return bench(dev_in, iters)


def kernel(context, embed_table, start_embed, Wp, bp, Wi, Wh, bh, Wo, bo, seed):
    global LAST_RESULTS
    run, _ = _get_runner()
    in_maps, Gp, bo_np = _prep_host(
        context, embed_table, start_embed, Wp, bp, Wi, Wh, bh, Wo, bo, seed
    )
    results, dev_in = run(in_maps)
    _CACHE["last_dev_in"] = dev_in
    res = type("R", (), {"results": results})()
    LAST_RESULTS = res

    logits = np.empty((B, T, V), np.float32)
    samples = np.empty((B, T), np.int32)
    for c in range(NCORES):
        lc = res.results[c]["louts"]  # (NTILES, 128, 2, T, V)
        # device z == lc + gumb bit-exactly -> samples match device feedback
        zc = lc + in_maps[c]["gumb"]
        sc = np.argmax(zc, axis=-1)  # (NTILES, 128, 2, T)
        sl = slice(c * BLOC, (c + 1) * BLOC)
        logits[sl] = lc.transpose(0, 2, 1, 3, 4).reshape(BLOC, T, V)
        samples[sl] = sc.transpose(0, 2, 1, 3).reshape(BLOC, T).astype(np.int32)
    logits += bo_np[None, None, :]
    return logits, samples


# revision 30
# speedup vs baseline: 1.0440x; 1.0440x over previous
"""Trainium2 Bass kernel for nn_BeliefDecoder (LSTM decoder with categorical
sampling), data-parallel over 8 NeuronCores.

Contract: kernel(**inputs) takes FULL unsharded inputs (as produced by
setup_inputs()) and returns the FULL output tuple
(logits (B, 15, 26) f32, samples (B, 15) int32).

Strategy
--------
- Pure data parallel: batch 65536 -> 8 cores x 8192 rows; weights replicated.
- Sampling must be bit-identical to jax.random.categorical: Gumbel noise is
  precomputed on host (CPU jax, threefry -> bit-exact) and shipped to the
  device. jax.random.categorical(k, logits) == argmax(gumbel(k, shape) +
  logits).
- On device, everything runs in a transposed layout [feature/vocab partitions,
  batch free]:
    h0 = tanh(ctx @ Wp + bp)                      (PE + ACT)
    per step: gates = Wh.h + EW'[tok] + xc + b    (PE accumulate + DVE add)
              i,f,g,o nonlinearities              (ACT)
              c,h update                          (DVE)
              logits = Wo.h                       (PE)
              z = logits + G[t]                   (DVE)
              m = max over vocab partitions       (GPSIMD partition_all_reduce)
              onehot = (z == m)                   (DVE is_equal)
  where xc = ctx @ Wi[:256] is precomputed once per batch-tile (context is
  time-invariant) and EW' = embed_table @ Wi[256:] + bh folds the embedding
  lookup into a tiny 26-row matmul against the onehot (the sampled token feeds
  back without ever materialising embeddings).
- The samples themselves are recovered on the host from the logits the kernel
  already outputs: argmax(logits + G) in f32 is bit-identical to the device's
  (z == m) selection.
- Matmuls run in true fp32 (4-pass H/L) so the sampled trajectory tracks the
  f32 reference closely enough that argmax flips are rare.
"""

import os
import numpy as np

H = 256          # hidden
T = 15           # decode steps (num_components)
V = 26           # vocab
E = 64           # embed dim
B = 65536        # batch
NCORES = 8
BT = 256         # batch tile (moving free dim per matmul)
BLOC = B // NCORES
NTILES = BLOC // BT  # 32

_CACHE = {}
LAST_RESULTS = None


def build_nc(n_tiles, use_f32r=False):
    """Build the Bass/Tile program for one core handling n_tiles*BT rows."""
    import concourse.bass as bass
    import concourse.tile as tile
    import concourse.mybir as mybir
    from concourse import bacc

    f32 = mybir.dt.float32
    wdt = mybir.dt.float32r if use_f32r else f32
    Sig = mybir.ActivationFunctionType.Sigmoid
    Tanh = mybir.ActivationFunctionType.Tanh
    ADD = mybir.AluOpType.add
    ISEQ = mybir.AluOpType.is_equal
    bloc = n_tiles * BT

    nc = bacc.Bacc("TRN2", target_bir_lowering=False, debug=False)

    ctxT = nc.dram_tensor("ctxT", (H, bloc), wdt, kind="ExternalInput")
    gumb = nc.dram_tensor("gumb", (n_tiles, 128, 2, T, V), f32, kind="ExternalInput")
    wp_d = nc.dram_tensor("wp", (H, H), wdt, kind="ExternalInput")
    wi_d = nc.dram_tensor("wi1", (H, 4 * H), wdt, kind="ExternalInput")
    wh_d = nc.dram_tensor("wh", (H, 4 * H), wdt, kind="ExternalInput")
    wo_d = nc.dram_tensor("wo", (H, V), wdt, kind="ExternalInput")
    ew_d = nc.dram_tensor("ew", (V, 4 * H), wdt, kind="ExternalInput")
    cst_d = nc.dram_tensor("consts", (128, 10), f32, kind="ExternalInput")
    id_d = nc.dram_tensor("ident", (128, 128), f32, kind="ExternalInput")
    louts = nc.dram_tensor(
        "louts", (n_tiles, 128, 2, T, V), f32, kind="ExternalOutput"
    )

    def mm(out, lhsT, rhs, family, **kw):
        nc.tensor.matmul(out, lhsT, rhs, **kw)

    with tile.TileContext(nc) as tc:
        with (
            tc.tile_pool(name="weights", bufs=1) as wpool,
            tc.tile_pool(name="state", bufs=3) as spool,
            tc.tile_pool(name="work", bufs=2) as kpool,
            tc.tile_pool(name="oh", bufs=4) as ohpool,
            tc.tile_pool(name="stage", bufs=2) as stpool,
            tc.tile_pool(name="pgates", bufs=1, space="PSUM") as pg_pool,
            tc.tile_pool(name="plog", bufs=1, space="PSUM") as pl_pool,
            tc.tile_pool(name="pbc", bufs=1, space="PSUM") as pbc_pool,
            tc.tile_pool(name="psetup", bufs=1, space="PSUM") as ps_pool,
        ):
            # ---- load weights (once) ----
            wp_sb = wpool.tile([128, 2, 2, 128], wdt, tag="wp")
            nc.sync.dma_start(
                wp_sb[:], wp_d.rearrange("(k p) (m c) -> p k m c", p=128, c=128)
            )
            wi_sb = wpool.tile([128, 2, 8, 128], wdt, tag="wi")
            nc.sync.dma_start(
                wi_sb[:], wi_d.rearrange("(k p) (m c) -> p k m c", p=128, c=128)
            )
            wh_sb = wpool.tile([128, 2, 8, 128], wdt, tag="wh")
            nc.sync.dma_start(
                wh_sb[:], wh_d.rearrange("(k p) (m c) -> p k m c", p=128, c=128)
            )
            wo_sb = wpool.tile([128, 2, V], wdt, tag="wo")
            nc.sync.dma_start(wo_sb[:], wo_d.rearrange("(k p) v -> p k v", p=128))
            ew_sb = wpool.tile([V, 8, 128], wdt, tag="ew")
            nc.sync.dma_start(ew_sb[:], ew_d.rearrange("v (m c) -> v m c", c=128))
            cst = wpool.tile([128, 10], f32, tag="cst")
            nc.sync.dma_start(cst[:], cst_d[:, :])
            id_sb = wpool.tile([128, 128], f32, tag="ident")
            nc.sync.dma_start(id_sb[:], id_d[:, :])

            state = {}  # per-tile persistent tiles
            prev_oh = {}

            def setup(j):
                ctx_sb = kpool.tile([128, 2, BT], wdt, tag="ctx")
                nc.sync.dma_start(
                    ctx_sb[:],
                    ctxT.rearrange("(k p) b -> p k b", p=128)[
                        :, :, j * BT : (j + 1) * BT
                    ],
                )
                g_sb = stpool.tile([128, 2, T, V], f32, tag="gum")
                nc.sync.dma_start(g_sb[:], gumb[j])

                # h0 = tanh(ctx @ Wp + bp)
                hT = spool.tile([128, 2, BT], wdt, tag="h")
                cT = spool.tile([128, 2, BT], f32, tag="c")
                nc.vector.memset(cT[:], 0.0)
                p_wp = ps_pool.tile([128, 2, BT], f32, tag="psetup")
                for m in range(2):
                    for k in range(2):
                        mm(
                            p_wp[:, m, :], wp_sb[:, k, m, :], ctx_sb[:, k, :],
                            "wp", start=(k == 0), stop=(k == 1),
                        )
                for m in range(2):
                    nc.scalar.activation(
                        hT[:, m, :], p_wp[:, m, :], Tanh, bias=cst[:, 8 + m : 9 + m]
                    )

                # xc = ctx @ Wi[:256]  (time-invariant gate contribution)
                xc = spool.tile([128, 8, BT], f32, tag="xc")
                for half in range(2):
                    p_xc = ps_pool.tile([128, 4, BT], f32, tag="psetup")
                    for mm_ in range(4):
                        m = half * 4 + mm_
                        for k in range(2):
                            mm(
                                p_xc[:, mm_, :], wi_sb[:, k, m, :], ctx_sb[:, k, :],
                                "xc", start=(k == 0), stop=(k == 1),
                            )
                    nc.scalar.copy(xc[:, half * 4 : half * 4 + 4, :], p_xc[:])

                lstage = stpool.tile([128, 2, T, V], f32, tag="lst")
                state[j] = (hT, cT, xc, g_sb, lstage)

            def step(j, t):
                hT, cT, xc, g_sb, lstage = state[j]
                # gates = Wh.h (+ EW'[tok_prev]) ; psum bank b holds gate b
                p_g = pg_pool.tile([128, 8, BT], f32, tag="pg")
                for gate in range(4):
                    for c2 in range(2):
                        m = gate * 2 + c2
                        for k in range(2):
                            mm(
                                p_g[:, m, :], wh_sb[:, k, m, :], hT[:, k, :],
                                "wh", start=(c2 == 0 and k == 0),
                                stop=(t == 0 and c2 == 1 and k == 1),
                            )
                if t > 0:
                    oh = prev_oh[j]
                    for gate in range(4):
                        for c2 in range(2):
                            m = gate * 2 + c2
                            mm(
                                p_g[:, m, :], ew_sb[:, m, :], oh[:],
                                "ew", start=False, stop=(c2 == 1),
                            )

                # gates += xc (+ bias); then nonlinearities
                gsb = kpool.tile([128, 8, BT], f32, tag="gsb")
                if t == 0:
                    # bias0 = bh + start_embed @ Wi[256:]
                    for ch in range(8):
                        nc.vector.scalar_tensor_tensor(
                            gsb[:, ch, :], p_g[:, ch, :], cst[:, ch : ch + 1],
                            xc[:, ch, :], op0=ADD, op1=ADD,
                        )
                else:
                    nc.vector.tensor_add(gsb[:], p_g[:], xc[:])

                acts = kpool.tile([128, 8, BT], f32, tag="acts")
                for gate in range(4):
                    nc.scalar.activation(
                        acts[:, 2 * gate : 2 * gate + 2, :],
                        gsb[:, 2 * gate : 2 * gate + 2, :],
                        Tanh if gate == 2 else Sig,
                    )

                # c = sig(f)*c + sig(i)*tanh(g);  h = sig(o)*tanh(c)
                t1 = kpool.tile([128, 2, BT], f32, tag="t1")
                t2 = kpool.tile([128, 2, BT], f32, tag="t2")
                nc.vector.tensor_mul(t1[:], acts[:, 2:4, :], cT[:])
                nc.vector.tensor_mul(t2[:], acts[:, 0:2, :], acts[:, 4:6, :])
                nc.vector.tensor_add(cT[:], t1[:], t2[:])
                tct = kpool.tile([128, 2, BT], f32, tag="tct")
                nc.scalar.activation(tct[:], cT[:], Tanh)
                nc.vector.tensor_mul(hT[:], acts[:, 6:8, :], tct[:])

                # logits in BATCH-major: out[batch 128, 26] via hT-as-stationary
                p_lb = pl_pool.tile([128, 2, V], f32, tag="plb")
                for c in range(2):
                    for k in range(2):
                        mm(
                            p_lb[:, c, :],
                            hT[:, k, c * 128 : (c + 1) * 128],
                            wo_sb[:, k, :],
                            "wo", start=(k == 0), stop=(k == 1),
                        )
                nc.scalar.copy(lstage[:, :, t, :], p_lb[:])

                if t < T - 1:
                    # z = logits + gumbel'; argmax along the free (vocab) dim;
                    # onehot back to vocab-major via PE transpose for feedback.
                    zt = kpool.tile([128, 2, V], f32, tag="zt")
                    nc.vector.tensor_add(zt[:], p_lb[:], g_sb[:, :, t, :])
                    mt = kpool.tile([128, 2], f32, tag="mt")
                    nc.vector.tensor_reduce(
                        mt[:], zt[:], axis=mybir.AxisListType.X,
                        op=mybir.AluOpType.max,
                    )
                    eqt = kpool.tile([128, 2, V], f32, tag="eqt")
                    for c in range(2):
                        nc.vector.tensor_scalar(
                            eqt[:, c, :], zt[:, c, :], mt[:, c : c + 1], None,
                            op0=ISEQ,
                        )
                    p_oh = pbc_pool.tile([V, 2, 128], f32, tag="poh")
                    for c in range(2):
                        nc.tensor.transpose(p_oh[:, c, :], eqt[:, c, :], id_sb[:])
                    oh = ohpool.tile([V, BT], wdt, tag="oh")
                    nc.scalar.copy(oh[:], p_oh[:])
                    prev_oh[j] = oh

            def finish(j):
                lstage = state[j][4]
                nc.sync.dma_start(louts[j], lstage[:])
                del state[j]
                prev_oh.pop(j, None)

            for jj in range(0, n_tiles, 2):
                pair = [jj] if jj + 1 >= n_tiles else [jj, jj + 1]
                for j in pair:
                    setup(j)
                for t in range(T):
                    for j in pair:
                        step(j, t)
                for j in pair:
                    finish(j)

    nc.compile()
    return nc


def _prep_host(context, embed_table, start_embed, Wp, bp, Wi, Wh, bh, Wo, bo, seed):
    """Host-side preprocessing: gumbel noise, weight repacks, per-core shards."""
    import jax
    import jax.numpy as jnp

    cpu = jax.devices("cpu")[0]
    with jax.default_device(cpu):
        keys = jax.random.split(jax.random.key(int(seed)), T)
        # per-key gumbel calls: bit-exact with jax.random.categorical's
        # internal noise (NOTE: vmap over keys yields different bits!)
        gfn = jax.jit(lambda k: jax.random.gumbel(k, (B, V), jnp.float32))
        G = np.stack([np.asarray(gfn(keys[t])) for t in range(T)])  # (T, B, V)
    bo = np.asarray(bo, np.float32)
    Gp = (G + bo[None, None, :]).astype(np.float32)  # z = logits + (gumbel + bo)

    Wi = np.asarray(Wi)
    bh = np.asarray(bh)
    ew = (
        np.asarray(embed_table).astype(np.float64) @ Wi[H:].astype(np.float64)
        + bh.astype(np.float64)
    ).astype(np.float32)  # (V, 4H)
    bias0 = (
        np.asarray(start_embed).astype(np.float64) @ Wi[H:].astype(np.float64)
        + bh.astype(np.float64)
    ).astype(np.float32)  # (4H,)
    consts = np.zeros((128, 10), np.float32)
    for ch in range(8):
        consts[:, ch] = bias0[ch * 128 : (ch + 1) * 128]
    bp = np.asarray(bp, np.float32)
    consts[:, 8] = bp[:128]
    consts[:, 9] = bp[128:]

    context = np.asarray(context, np.float32)
    in_maps = []
    for c in range(NCORES):
        sl = slice(c * BLOC, (c + 1) * BLOC)
        ctxT = np.ascontiguousarray(context[sl].T)  # (H, BLOC)
        # (T, BLOC, V) -> (NTILES, 128, 2, T, V); b = j*256 + c*128 + p
        gc = np.ascontiguousarray(
            Gp[:, sl, :].reshape(T, NTILES, 2, 128, V).transpose(1, 3, 2, 0, 4)
        )
        in_maps.append(
            dict(
                ctxT=ctxT,
                gumb=gc,
                wp=np.asarray(Wp, np.float32),
                wi1=Wi[:H].astype(np.float32),
                wh=np.asarray(Wh, np.float32),
                wo=np.asarray(Wo, np.float32),
                ew=ew,
                consts=consts,
                ident=np.eye(128, dtype=np.float32),
            )
        )
    return in_maps, Gp, bo


def _get_runner():
    """Build (once) a cached sharded-PJRT executable for the kernel, modeled on
    concourse.bass2jax.run_bass_via_pjrt's multi-core path, plus a bench()
    that times steady-state execution with device-resident inputs."""
    if "runner" in _CACHE:
        return _CACHE["runner"]
    import jax
    import jax.numpy as jnp
    from jax.sharding import Mesh, PartitionSpec, NamedSharding
    from jax.experimental.shard_map import shard_map
    import concourse.mybir as mybir
    from concourse import bass2jax

    use_f32r = os.environ.get("TRN_F32R", "0") == "1"
    nc = _CACHE.get(("nc", NTILES, use_f32r))
    if nc is None:
        nc = _CACHE[("nc", NTILES, use_f32r)] = build_nc(NTILES, use_f32r)
    bass2jax.install_neuronx_cc_hook()

    partition_name = (
        nc.partition_id_tensor.name if nc.partition_id_tensor else None
    )
    in_names, out_names, out_avals, zero_shapes = [], [], [], []
    for alloc in nc.m.functions[0].allocations:
        if not isinstance(alloc, mybir.MemoryLocationSet):
            continue
        name = alloc.memorylocations[0].name
        if alloc.kind == "ExternalInput":
            if name != partition_name:
                in_names.append(name)
        elif alloc.kind == "ExternalOutput":
            shape = tuple(alloc.tensor_shape)
            dtype = mybir.dt.np(alloc.dtype)
            out_names.append(name)
            out_avals.append(jax.core.ShapedArray(shape, dtype))
            zero_shapes.append((shape, dtype))
    n_params = len(in_names)
    n_outs = len(out_names)
    all_names = tuple(
        in_names + out_names + ([partition_name] if partition_name else [])
    )

    def _body(*args):
        operands = list(args)
        if partition_name is not None:
            operands.append(bass2jax.partition_id_tensor())
        outs = bass2jax._bass_exec_p.bind(
            *operands,
            out_avals=tuple(out_avals),
            in_names=all_names,
            out_names=tuple(out_names),
            lowering_input_output_aliases=(),
            sim_require_finite=True,
            sim_require_nnan=True,
            nc=nc,
        )
        return tuple(outs)

    devices = jax.devices()[:NCORES]
    mesh = Mesh(np.asarray(devices), ("core",))
    pspec = PartitionSpec("core")
    sharded = jax.jit(
        shard_map(
            _body,
            mesh=mesh,
            in_specs=(pspec,) * (n_params + n_outs),
            out_specs=(pspec,) * n_outs,
            check_rep=False,
        ),
        donate_argnums=tuple(range(n_params, n_params + n_outs)),
        keep_unused=True,
    )
    zeros_fn = jax.jit(
        lambda: tuple(
            jnp.zeros((NCORES * s[0], *s[1:]), d) for s, d in zero_shapes
        ),
        out_shardings=tuple(NamedSharding(mesh, pspec) for _ in zero_shapes),
    )

    def run(in_maps):
        concat_in = [
            np.concatenate([m[name] for m in in_maps], axis=0) for name in in_names
        ]
        dev_in = [jax.device_put(a, NamedSharding(mesh, pspec)) for a in concat_in]
        out = sharded(*dev_in, *zeros_fn())
        results = []
        for c in range(NCORES):
            results.append(
                {
                    name: np.asarray(out[i]).reshape(NCORES, *out_avals[i].shape)[c]
                    for i, name in enumerate(out_names)
                }
            )
        return results, dev_in

    def bench(dev_in, iters=3):
        import time

        times = []
        for _ in range(iters):
            zs = jax.block_until_ready(zeros_fn())
            t0 = time.perf_counter()
            out = sharded(*dev_in, *zs)
            jax.block_until_ready(out)
            times.append(time.perf_counter() - t0)
        return times

    _CACHE["runner"] = (run, bench)
    return _CACHE["runner"]


def bench_exec(iters=3):
    """Steady-state wall time per execution (device-resident inputs)."""
    run, bench = _get_runner()
    dev_in = _CACHE.get("last_dev_in")
    if dev_in is None:
        raise RuntimeError("call kernel() first")
    return bench(dev_in, iters)


def kernel(context, embed_table, start_embed, Wp, bp, Wi, Wh, bh, Wo, bo, seed):
    global LAST_RESULTS
    run, _ = _get_runner()
    in_maps, Gp, bo_np = _prep_host(
        context, embed_table, start_embed, Wp, bp, Wi, Wh, bh, Wo, bo, seed
    )
    results, dev_in = run(in_maps)
    _CACHE["last_dev_in"] = dev_in
    res = type("R", (), {"results": results})()
    LAST_RESULTS = res

    logits = np.empty((B, T, V), np.float32)
    samples = np.empty((B, T), np.int32)
    for c in range(NCORES):
        lc = res.results[c]["louts"]  # (NTILES, 128, 2, T, V)
        # device z == lc + gumb bit-exactly -> samples match device feedback
        zc = lc + in_maps[c]["gumb"]
        sc = np.argmax(zc, axis=-1)  # (NTILES, 128, 2, T)
        sl = slice(c * BLOC, (c + 1) * BLOC)
        logits[sl] = lc.transpose(0, 2, 1, 3, 4).reshape(BLOC, T, V)
        samples[sl] = sc.transpose(0, 2, 1, 3).reshape(BLOC, T).astype(np.int32)
    logits += bo_np[None, None, :]
    return logits, samples
